# revision 7
# baseline (speedup 1.0000x reference)
"""Trainium2 Bass kernel for the EnsembleDynamicsNetwork problem.

Strategy:
- Ensemble-sharding: member e -> NeuronCore e (8 members, 8 cores). Every core
  sees the full batch; members are fully independent (no collectives).
- Input normalization is folded into layer-1 weights/bias on the host;
  output denormalization of the means is folded into the output head weights.
  denorm_stds = exp(clip(ls, -10, .5)) * dstd = exp(clip(ls, -10, .5) + log(dstd)),
  computed with the ACT engine's fused exp(x + bias).
- On-chip activations live transposed: A_l = h_l.T stored as 4x [128, 512chunk]
  SBUF tiles, so every layer is out[c] = sum_k W[kblk, cblk].T-free matmuls with
  plain weight blocks as the stationary operand and NO inter-layer transposes.
- Matmuls run in float32r (full-rate fp32 mode, ~1e-4 rel err), fp32 PSUM.
- Final [feat, batch] -> [batch, feat] layout fix via DVE 32x32 block transpose
  + strided DMA. rewards/dones stay transposed ([2, B]); host adds their scalar
  bias and reshapes.
"""
import sys

sys.path.insert(0, "/opt/trn_rl_repo")

import numpy as np

# --- problem constants (hardcoded; kernel.py must be self-contained) ---
E = 8
H = 512
OBS = 32
ACT_DIM = 16
D_IN = OBS + ACT_DIM  # 48
B = 32768
LOG_STD_MIN = -10.0
LOG_STD_MAX = 0.5
N_CORES = 8
CHUNK = 512
N_CHUNKS = B // CHUNK  # 64
KBLK = H // 128  # 4
OUTW = OBS + OBS + 1 + 1  # 66

_cache = {}


def _build_program():
    import concourse.bass as bass
    import concourse.mybir as mybir
    import concourse.tile as tile
    from concourse import bacc

    F32 = mybir.dt.float32
    F32R = mybir.dt.float32r
    AF = mybir.ActivationFunctionType
    ALU = mybir.AluOpType

    nc = bacc.Bacc("TRN2", target_bir_lowering=False, debug=False)

    xt = nc.dram_tensor("xt", [D_IN, B], F32R, kind="ExternalInput")
    w1 = nc.dram_tensor("w1", [D_IN, H], F32R, kind="ExternalInput")
    w2 = nc.dram_tensor("w2", [128, KBLK, H], F32R, kind="ExternalInput")
    w3 = nc.dram_tensor("w3", [128, KBLK, H], F32R, kind="ExternalInput")
    w4 = nc.dram_tensor("w4", [128, KBLK, H], F32R, kind="ExternalInput")
    wo = nc.dram_tensor("wo", [128, KBLK, OUTW], F32R, kind="ExternalInput")
    bh = nc.dram_tensor("bh", [128, 16], F32, kind="ExternalInput")
    bo = nc.dram_tensor("bo", [64, 1], F32, kind="ExternalInput")
    lds = nc.dram_tensor("lds", [64, 1], F32, kind="ExternalInput")

    om = nc.dram_tensor("om", [B, OBS], F32, kind="ExternalOutput")
    os_ = nc.dram_tensor("os", [B, OBS], F32, kind="ExternalOutput")
    ord_ = nc.dram_tensor("ord", [2, B], F32, kind="ExternalOutput")

    with tile.TileContext(nc) as tc:
        with (
            tc.tile_pool(name="wp", bufs=1) as wp,
            tc.tile_pool(name="a0p", bufs=6) as a0p,
            tc.tile_pool(name="ap", bufs=20) as ap,
            tc.tile_pool(name="op", bufs=3) as op,
            tc.tile_pool(name="php", bufs=5, space="PSUM") as php,
            tc.tile_pool(name="pop", bufs=2, space="PSUM") as pop,
        ):
            w1_sb = wp.tile([D_IN, H], F32R, tag="w1")
            w2_sb = wp.tile([128, KBLK, H], F32R, tag="w2")
            w3_sb = wp.tile([128, KBLK, H], F32R, tag="w3")
            w4_sb = wp.tile([128, KBLK, H], F32R, tag="w4")
            wo_sb = wp.tile([128, KBLK, OUTW], F32R, tag="wo")
            bh_sb = wp.tile([128, 16], F32, tag="bh")
            bo_sb = wp.tile([64, 1], F32, tag="bo")
            lds_sb = wp.tile([64, 1], F32, tag="lds")
            nc.sync.dma_start(w1_sb[:], w1[:])
            nc.sync.dma_start(w2_sb[:], w2[:])
            nc.sync.dma_start(w3_sb[:], w3[:])
            nc.sync.dma_start(w4_sb[:], w4[:])
            nc.sync.dma_start(wo_sb[:], wo[:])
            nc.sync.dma_start(bh_sb[:], bh[:])
            nc.sync.dma_start(bo_sb[:], bo[:])
            nc.sync.dma_start(lds_sb[:], lds[:])

            w_mid = [w2_sb, w3_sb, w4_sb]

            def relu_into(dst, src, bias_ap, use_act):
                """dst = relu(src + bias), on ACT or DVE."""
                if use_act:
                    nc.scalar.activation(dst, src, AF.Relu, bias=bias_ap)
                else:
                    nc.vector.tensor_scalar(
                        out=dst, in0=src, scalar1=bias_ap, scalar2=0.0,
                        op0=ALU.add, op1=ALU.max,
                    )

            for i in range(N_CHUNKS):
                cs = bass.ts(i, CHUNK)
                a0 = a0p.tile([D_IN, CHUNK], F32R, tag="a0")
                nc.sync.dma_start(a0[:], xt[:, cs])

                # layer 1: K = 48, single matmul per output block
                a_prev = []
                for c in range(KBLK):
                    ph = php.tile([128, CHUNK], F32, tag="ph")
                    nc.tensor.matmul(
                        ph[:], w1_sb[:, bass.ts(c, 128)], a0[:],
                        start=True, stop=True,
                    )
                    a = ap.tile([128, CHUNK], F32R, tag="a")
                    relu_into(a[:], ph[:], bh_sb[:, c : c + 1], use_act=(c < 2))
                    a_prev.append(a)

                # layers 2..4: K = 512 in 4 blocks accumulated in PSUM.
                # k-major issue order: the first 4 matmuls of a layer read only
                # a_prev[0] (the oldest tile), burying the relu latency of the
                # last-produced tile instead of stalling on it.
                for li, w_sb in enumerate(w_mid):
                    phs = [
                        php.tile([128, CHUNK], F32, tag="ph", name=f"ph_{i}_{li}_{c}")
                        for c in range(KBLK)
                    ]
                    for k in range(KBLK):
                        for c in range(KBLK):
                            nc.tensor.matmul(
                                phs[c][:], w_sb[:, k, bass.ts(c, 128)], a_prev[k][:],
                                start=(k == 0), stop=(k == KBLK - 1),
                            )
                    a_next = []
                    for c in range(KBLK):
                        a = ap.tile([128, CHUNK], F32R, tag="a")
                        bias_ap = bh_sb[:, 4 * (li + 1) + c : 4 * (li + 1) + c + 1]
                        relu_into(a[:], phs[c][:], bias_ap, use_act=(c < 2))
                        a_next.append(a)
                    a_prev = a_next

                # output head: [66, 512] = rows 0:32 means, 32:64 logstds, 64:66 rew/done
                po = pop.tile([OUTW, CHUNK], F32, tag="po")
                for k in range(KBLK):
                    nc.tensor.matmul(
                        po[:], wo_sb[:, k, :], a_prev[k][:],
                        start=(k == 0), stop=(k == KBLK - 1),
                    )

                t_out = op.tile([64, CHUNK], F32, tag="t_out")
                t_cl = op.tile([64, CHUNK], F32, tag="t_cl")
                t_mn = op.tile([64, CHUNK], F32, tag="t_mn")
                # means: + bias (denorm already folded into weights) — on ACT
                nc.scalar.activation(
                    t_out[0:32, :], po[0:32, :], AF.Identity,
                    bias=bo_sb[0:32, :],
                )
                # logstds: + bias, clip to [-10, 0.5], then exp(x + log(dstd))
                nc.vector.tensor_scalar(
                    out=t_cl[32:64, :], in0=po[32:64, :],
                    scalar1=bo_sb[32:64, :], scalar2=LOG_STD_MIN,
                    op0=ALU.add, op1=ALU.max,
                )
                nc.gpsimd.tensor_scalar_min(
                    out=t_mn[32:64, :], in0=t_cl[32:64, :], scalar1=LOG_STD_MAX,
                )
                nc.scalar.activation(
                    t_out[32:64, :], t_mn[32:64, :], AF.Exp,
                    bias=lds_sb[32:64, :],
                )
                # block-transpose [feat, batch] -> batch-major and store
                t_tr = op.tile([64, CHUNK], F32, tag="t_tr")
                nc.vector.transpose(t_tr[:], t_out[:])
                nc.sync.dma_start(
                    om[cs, :].rearrange("(j p) q -> p j q", p=32),
                    t_tr[0:32, :].rearrange("p (j q) -> p j q", q=32),
                )
                nc.sync.dma_start(
                    os_[cs, :].rearrange("(j p) q -> p j q", p=32),
                    t_tr[32:64, :].rearrange("p (j q) -> p j q", q=32),
                )
                # rewards/dones raw (bias added on host), stay transposed.
                # DMA cannot read PSUM -> bounce through SBUF at matching
                # partition offset (engine lanes are partition-aligned).
                t_rd = op.tile([66, CHUNK], F32, tag="t_rd")
                nc.scalar.copy(t_rd[64:66, :], po[64:66, :])
                nc.sync.dma_start(ord_[:, cs], t_rd[64:66, :])

    nc.compile()
    return nc


def _prep_core_inputs(xt_full, W1, b1, W2, b2, W3, b3, W4, b4,
                      Wm, bm, Wls, bls, Wr, br, Wd, bd,
                      state_mean, state_std, action_mean, action_std,
                      delta_mean, delta_std, e):
    f64 = np.float64
    mu = np.concatenate([state_mean, action_mean]).astype(f64)
    sig = np.concatenate([state_std, action_std]).astype(f64)

    W1e = W1[e].astype(f64)
    w1f = W1e / sig[:, None]
    b1f = b1[e].astype(f64) - (mu / sig) @ W1e

    def blocks(w, width):
        return np.ascontiguousarray(
            w.reshape(KBLK, 128, width).transpose(1, 0, 2)
        ).astype(np.float32)

    dstd = delta_std.astype(f64)
    dmean = delta_mean.astype(f64)
    wm_f = Wm[e].astype(f64) * dstd[None, :]
    bo_v = np.concatenate([bm[e].astype(f64) * dstd + dmean, bls[e]]).astype(np.float32)
    wo_full = np.concatenate(
        [wm_f.astype(np.float32), Wls[e], Wr[e], Wd[e]], axis=1
    )  # [H, 66]

    bh_v = np.zeros((128, 16), np.float32)
    for l, bl in enumerate([b1f.astype(np.float32), b2[e], b3[e], b4[e]]):
        for c in range(KBLK):
            bh_v[:, 4 * l + c] = bl[c * 128 : (c + 1) * 128]

    lds_v = np.zeros((64, 1), np.float32)
    lds_v[32:64, 0] = np.log(dstd).astype(np.float32)

    return {
        "xt": xt_full,
        "w1": np.ascontiguousarray(w1f.astype(np.float32)),
        "w2": blocks(W2[e], H),
        "w3": blocks(W3[e], H),
        "w4": blocks(W4[e], H),
        "wo": blocks(wo_full, OUTW),
        "bh": bh_v,
        "bo": bo_v[:, None].copy(),
        "lds": lds_v,
    }


def kernel(states, actions, state_mean, state_std, action_mean, action_std,
           delta_mean, delta_std,
           W1, b1, W2, b2, W3, b3, W4, b4,
           Wm, bm, Wls, bls, Wr, br, Wd, bd, **run_kwargs):
    from concourse.bass_utils import run_bass_kernel_spmd

    to_np = lambda a: np.asarray(a, dtype=np.float32)
    states, actions = to_np(states), to_np(actions)
    args = [to_np(a) for a in (W1, b1, W2, b2, W3, b3, W4, b4,
                               Wm, bm, Wls, bls, Wr, br, Wd, bd)]
    (W1, b1, W2, b2, W3, b3, W4, b4,
     Wm, bm, Wls, bls, Wr, br, Wd, bd) = args
    norms = [to_np(a) for a in (state_mean, state_std, action_mean, action_std,
                                delta_mean, delta_std)]
    (state_mean, state_std, action_mean, action_std,
     delta_mean, delta_std) = norms

    if "nc" not in _cache:
        _cache["nc"] = _build_program()
    nc = _cache["nc"]

    xt_full = np.ascontiguousarray(
        np.concatenate([states, actions], axis=1).T
    ).astype(np.float32)

    in_maps = [
        _prep_core_inputs(xt_full, W1, b1, W2, b2, W3, b3, W4, b4,
                          Wm, bm, Wls, bls, Wr, br, Wd, bd,
                          state_mean, state_std, action_mean, action_std,
                          delta_mean, delta_std, e)
        for e in range(N_CORES)
    ]

    res = run_bass_kernel_spmd(nc, in_maps, list(range(N_CORES)), **run_kwargs)
    _cache["last_result"] = res

    means = np.stack([res.results[e]["om"] for e in range(N_CORES)])
    stds = np.stack([res.results[e]["os"] for e in range(N_CORES)])
    rd = np.stack([res.results[e]["ord"] for e in range(N_CORES)])  # [E, 2, B]
    rewards = (rd[:, 0, :] + br[:, 0:1]).astype(np.float32)[:, :, None]
    dones = (rd[:, 1, :] + bd[:, 0:1]).astype(np.float32)[:, :, None]
    return means, stds, rewards, dones


# revision 8
# speedup vs baseline: 1.5662x; 1.5662x over previous
"""Trainium2 Bass kernel for the EnsembleDynamicsNetwork problem.

Strategy:
- Ensemble-sharding: member e -> NeuronCore e (8 members, 8 cores). Every core
  sees the full batch; members are fully independent (no collectives).
- Input normalization is folded into layer-1 weights/bias on the host;
  output denormalization of the means is folded into the output head weights.
  denorm_stds = exp(clip(ls, -10, .5)) * dstd = exp(clip(ls, -10, .5) + log(dstd)),
  computed with the ACT engine's fused exp(x + bias).
- On-chip activations live transposed: A_l = h_l.T stored as 4x [128, 512chunk]
  SBUF tiles, so every layer is out[c] = sum_k W[kblk, cblk].T-free matmuls with
  plain weight blocks as the stationary operand and NO inter-layer transposes.
- Matmuls run in float32r (full-rate fp32 mode, ~1e-4 rel err), fp32 PSUM.
- Final [feat, batch] -> [batch, feat] layout fix via DVE 32x32 block transpose
  + strided DMA. rewards/dones stay transposed ([2, B]); host adds their scalar
  bias and reshapes.
"""
import sys

sys.path.insert(0, "/opt/trn_rl_repo")

import numpy as np

# --- problem constants (hardcoded; kernel.py must be self-contained) ---
E = 8
H = 512
OBS = 32
ACT_DIM = 16
D_IN = OBS + ACT_DIM  # 48
B = 32768
LOG_STD_MIN = -10.0
LOG_STD_MAX = 0.5
N_CORES = 8
CHUNK = 512
N_CHUNKS = B // CHUNK  # 64
KBLK = H // 128  # 4
OUTW = OBS + OBS + 1 + 1  # 66

_cache = {}


def _build_program():
    import concourse.bass as bass
    import concourse.mybir as mybir
    import concourse.tile as tile
    from concourse import bacc

    F32 = mybir.dt.float32
    F32R = mybir.dt.float32r
    AF = mybir.ActivationFunctionType
    ALU = mybir.AluOpType

    nc = bacc.Bacc("TRN2", target_bir_lowering=False, debug=False)

    xt = nc.dram_tensor("xt", [D_IN, B], F32R, kind="ExternalInput")
    w1 = nc.dram_tensor("w1", [D_IN, H], F32R, kind="ExternalInput")
    w2 = nc.dram_tensor("w2", [128, KBLK, H], F32R, kind="ExternalInput")
    w3 = nc.dram_tensor("w3", [128, KBLK, H], F32R, kind="ExternalInput")
    w4 = nc.dram_tensor("w4", [128, KBLK, H], F32R, kind="ExternalInput")
    wo = nc.dram_tensor("wo", [128, KBLK, OUTW], F32R, kind="ExternalInput")
    bh = nc.dram_tensor("bh", [128, 16], F32, kind="ExternalInput")
    bo = nc.dram_tensor("bo", [64, 1], F32, kind="ExternalInput")
    lds = nc.dram_tensor("lds", [64, 1], F32, kind="ExternalInput")

    om = nc.dram_tensor("om", [B, OBS], F32, kind="ExternalOutput")
    os_ = nc.dram_tensor("os", [B, OBS], F32, kind="ExternalOutput")
    ord_ = nc.dram_tensor("ord", [2, B], F32, kind="ExternalOutput")

    with tile.TileContext(nc) as tc:
        with (
            tc.tile_pool(name="wp", bufs=1) as wp,
            tc.tile_pool(name="a0p", bufs=6) as a0p,
            tc.tile_pool(name="ap", bufs=20) as ap,
            tc.tile_pool(name="op", bufs=3) as op,
            tc.tile_pool(name="php", bufs=5, space="PSUM") as php,
            tc.tile_pool(name="pop", bufs=2, space="PSUM") as pop,
        ):
            w1_sb = wp.tile([D_IN, H], F32R, tag="w1")
            w2_sb = wp.tile([128, KBLK, H], F32R, tag="w2")
            w3_sb = wp.tile([128, KBLK, H], F32R, tag="w3")
            w4_sb = wp.tile([128, KBLK, H], F32R, tag="w4")
            wo_sb = wp.tile([128, KBLK, OUTW], F32R, tag="wo")
            bh_sb = wp.tile([128, 16], F32, tag="bh")
            bo_sb = wp.tile([64, 1], F32, tag="bo")
            lds_sb = wp.tile([64, 1], F32, tag="lds")
            nc.sync.dma_start(w1_sb[:], w1[:])
            nc.sync.dma_start(w2_sb[:], w2[:])
            nc.sync.dma_start(w3_sb[:], w3[:])
            nc.sync.dma_start(w4_sb[:], w4[:])
            nc.sync.dma_start(wo_sb[:], wo[:])
            nc.sync.dma_start(bh_sb[:], bh[:])
            nc.sync.dma_start(bo_sb[:], bo[:])
            nc.sync.dma_start(lds_sb[:], lds[:])

            w_mid = [w2_sb, w3_sb, w4_sb]

            def relu_into(dst, src, bias_ap, use_act):
                """dst = relu(src + bias), on ACT or DVE."""
                if use_act:
                    nc.scalar.activation(dst, src, AF.Relu, bias=bias_ap)
                else:
                    nc.vector.tensor_scalar(
                        out=dst, in0=src, scalar1=bias_ap, scalar2=0.0,
                        op0=ALU.add, op1=ALU.max,
                    )

            for i in range(N_CHUNKS):
                cs = bass.ts(i, CHUNK)
                a0 = a0p.tile([D_IN, CHUNK], F32R, tag="a0")
                nc.sync.dma_start(a0[:], xt[:, cs])

                # layer 1: K = 48, single matmul per output block
                a_prev = []
                for c in range(KBLK):
                    ph = php.tile([128, CHUNK], F32, tag="ph")
                    nc.tensor.matmul(
                        ph[:], w1_sb[:, bass.ts(c, 128)], a0[:],
                        start=True, stop=True,
                    )
                    a = ap.tile([128, CHUNK], F32R, tag="a")
                    relu_into(a[:], ph[:], bh_sb[:, c : c + 1], use_act=(c < 2))
                    a_prev.append(a)

                # layers 2..4: K = 512 in 4 blocks accumulated in PSUM.
                # k-major issue order: the first 4 matmuls of a layer read only
                # a_prev[0] (the oldest tile), burying the relu latency of the
                # last-produced tile instead of stalling on it.
                for li, w_sb in enumerate(w_mid):
                    phs = [
                        php.tile([128, CHUNK], F32, tag="ph", name=f"ph_{i}_{li}_{c}")
                        for c in range(KBLK)
                    ]
                    for k in range(KBLK):
                        for c in range(KBLK):
                            nc.tensor.matmul(
                                phs[c][:], w_sb[:, k, bass.ts(c, 128)], a_prev[k][:],
                                start=(k == 0), stop=(k == KBLK - 1),
                            )
                    a_next = []
                    for c in range(KBLK):
                        a = ap.tile([128, CHUNK], F32R, tag="a")
                        bias_ap = bh_sb[:, 4 * (li + 1) + c : 4 * (li + 1) + c + 1]
                        relu_into(a[:], phs[c][:], bias_ap, use_act=(c < 2))
                        a_next.append(a)
                    a_prev = a_next

                # output head: [66, 512] = rows 0:32 means, 32:64 logstds, 64:66 rew/done
                po = pop.tile([OUTW, CHUNK], F32, tag="po")
                for k in range(KBLK):
                    nc.tensor.matmul(
                        po[:], wo_sb[:, k, :], a_prev[k][:],
                        start=(k == 0), stop=(k == KBLK - 1),
                    )

                t_out = op.tile([64, CHUNK], F32, tag="t_out")
                t_cl = op.tile([64, CHUNK], F32, tag="t_cl")
                t_mn = op.tile([64, CHUNK], F32, tag="t_mn")
                # means: + bias (denorm already folded into weights) — on ACT
                nc.scalar.activation(
                    t_out[0:32, :], po[0:32, :], AF.Identity,
                    bias=bo_sb[0:32, :],
                )
                # logstds: + bias, clip to [-10, 0.5], then exp(x + log(dstd))
                nc.vector.tensor_scalar(
                    out=t_cl[32:64, :], in0=po[32:64, :],
                    scalar1=bo_sb[32:64, :], scalar2=LOG_STD_MIN,
                    op0=ALU.add, op1=ALU.max,
                )
                nc.vector.tensor_scalar_min(
                    out=t_mn[32:64, :], in0=t_cl[32:64, :], scalar1=LOG_STD_MAX,
                )
                nc.scalar.activation(
                    t_out[32:64, :], t_mn[32:64, :], AF.Exp,
                    bias=lds_sb[32:64, :],
                )
                # block-transpose [feat, batch] -> batch-major and store
                t_tr = op.tile([64, CHUNK], F32, tag="t_tr")
                nc.vector.transpose(t_tr[:], t_out[:])
                nc.sync.dma_start(
                    om[cs, :].rearrange("(j p) q -> p j q", p=32),
                    t_tr[0:32, :].rearrange("p (j q) -> p j q", q=32),
                )
                nc.sync.dma_start(
                    os_[cs, :].rearrange("(j p) q -> p j q", p=32),
                    t_tr[32:64, :].rearrange("p (j q) -> p j q", q=32),
                )
                # rewards/dones raw (bias added on host), stay transposed.
                # DMA cannot read PSUM -> bounce through SBUF at matching
                # partition offset (engine lanes are partition-aligned).
                t_rd = op.tile([66, CHUNK], F32, tag="t_rd")
                nc.scalar.copy(t_rd[64:66, :], po[64:66, :])
                nc.sync.dma_start(ord_[:, cs], t_rd[64:66, :])

    nc.compile()
    return nc


def _prep_core_inputs(xt_full, W1, b1, W2, b2, W3, b3, W4, b4,
                      Wm, bm, Wls, bls, Wr, br, Wd, bd,
                      state_mean, state_std, action_mean, action_std,
                      delta_mean, delta_std, e):
    f64 = np.float64
    mu = np.concatenate([state_mean, action_mean]).astype(f64)
    sig = np.concatenate([state_std, action_std]).astype(f64)

    W1e = W1[e].astype(f64)
    w1f = W1e / sig[:, None]
    b1f = b1[e].astype(f64) - (mu / sig) @ W1e

    def blocks(w, width):
        return np.ascontiguousarray(
            w.reshape(KBLK, 128, width).transpose(1, 0, 2)
        ).astype(np.float32)

    dstd = delta_std.astype(f64)
    dmean = delta_mean.astype(f64)
    wm_f = Wm[e].astype(f64) * dstd[None, :]
    bo_v = np.concatenate([bm[e].astype(f64) * dstd + dmean, bls[e]]).astype(np.float32)
    wo_full = np.concatenate(
        [wm_f.astype(np.float32), Wls[e], Wr[e], Wd[e]], axis=1
    )  # [H, 66]

    bh_v = np.zeros((128, 16), np.float32)
    for l, bl in enumerate([b1f.astype(np.float32), b2[e], b3[e], b4[e]]):
        for c in range(KBLK):
            bh_v[:, 4 * l + c] = bl[c * 128 : (c + 1) * 128]

    lds_v = np.zeros((64, 1), np.float32)
    lds_v[32:64, 0] = np.log(dstd).astype(np.float32)

    return {
        "xt": xt_full,
        "w1": np.ascontiguousarray(w1f.astype(np.float32)),
        "w2": blocks(W2[e], H),
        "w3": blocks(W3[e], H),
        "w4": blocks(W4[e], H),
        "wo": blocks(wo_full, OUTW),
        "bh": bh_v,
        "bo": bo_v[:, None].copy(),
        "lds": lds_v,
    }


def kernel(states, actions, state_mean, state_std, action_mean, action_std,
           delta_mean, delta_std,
           W1, b1, W2, b2, W3, b3, W4, b4,
           Wm, bm, Wls, bls, Wr, br, Wd, bd, **run_kwargs):
    from concourse.bass_utils import run_bass_kernel_spmd

    to_np = lambda a: np.asarray(a, dtype=np.float32)
    states, actions = to_np(states), to_np(actions)
    args = [to_np(a) for a in (W1, b1, W2, b2, W3, b3, W4, b4,
                               Wm, bm, Wls, bls, Wr, br, Wd, bd)]
    (W1, b1, W2, b2, W3, b3, W4, b4,
     Wm, bm, Wls, bls, Wr, br, Wd, bd) = args
    norms = [to_np(a) for a in (state_mean, state_std, action_mean, action_std,
                                delta_mean, delta_std)]
    (state_mean, state_std, action_mean, action_std,
     delta_mean, delta_std) = norms

    if "nc" not in _cache:
        _cache["nc"] = _build_program()
    nc = _cache["nc"]

    xt_full = np.ascontiguousarray(
        np.concatenate([states, actions], axis=1).T
    ).astype(np.float32)

    in_maps = [
        _prep_core_inputs(xt_full, W1, b1, W2, b2, W3, b3, W4, b4,
                          Wm, bm, Wls, bls, Wr, br, Wd, bd,
                          state_mean, state_std, action_mean, action_std,
                          delta_mean, delta_std, e)
        for e in range(N_CORES)
    ]

    res = run_bass_kernel_spmd(nc, in_maps, list(range(N_CORES)), **run_kwargs)
    _cache["last_result"] = res

    means = np.stack([res.results[e]["om"] for e in range(N_CORES)])
    stds = np.stack([res.results[e]["os"] for e in range(N_CORES)])
    rd = np.stack([res.results[e]["ord"] for e in range(N_CORES)])  # [E, 2, B]
    rewards = (rd[:, 0, :] + br[:, 0:1]).astype(np.float32)[:, :, None]
    dones = (rd[:, 1, :] + bd[:, 0:1]).astype(np.float32)[:, :, None]
    return means, stds, rewards, dones
